# revision 27
# baseline (speedup 1.0000x reference)
"""Trainium2 Bass kernel for the neural 2D min-sum LDPC decoder problem.

Strategy (v5)
-------------
Data-parallel over the batch: B=512 codewords, 64 per NeuronCore (8 cores).
Per core, per-edge state lives in SBUF with the graph on the partition axis
(check c <-> partition c%128, block c//128) and the 64-batch on the free
axis (256B rows).  Variables are relabeled by their slot-{0,1} (layer-0)
position so u / llr storage row = (parity, check-row) of the layer-0 edge.

Both per-iteration crossings pipeline with compute at 4-block granularity:

  crossing 1 (c2v -> per-variable sums): SBUF->SBUF dma_scatter_add in
      parity-split CCE mode (sbuf_tokens_per_rank=128).  Slot plane 2+j
      scatter-adds into its own accumulator pair SA[j] on queue j (4
      independent WAW chains ride 4 SWDGE queues); a 512-descriptor wave
      fires after every check compute chunk, so the chains drain in
      lockstep with compute.  dest code = ((g*2+parity)<<7) | p.
  u-compute   u = llr + alpha*(SA0+SA1+SA2+SA3 + c2v_l0), llr streamed
      from DRAM; u written to udram (affine HWDGE).
  crossing 2 (u -> slot positions 2..5): destination-chunked HBM gathers
      from udram in 512-descriptor waves; wave k unblocks check chunk k of
      the next iteration while later waves drain underneath its compute.

The SWDGE descriptor drain (~3ns/desc pipelined, ~12ns/desc on a WAW
chain) is the capacity limit: 32768 descriptors x 256B per iteration.
alpha/beta are baked as immediates (compiled after inputs are known).
"""

import sys

for _p in ("/opt/trn_rl_repo",):
    if _p not in sys.path:
        sys.path.insert(0, _p)

import numpy as np

import concourse.bass as bass
import concourse.bacc as bacc
import concourse.mybir as mybir
import concourse.tile as tile
from concourse.bass_utils import run_bass_kernel_spmd

N = 8192          # variable nodes
M = 4096          # check nodes
DC = 6            # check degree (slots)
DV = 3            # variable degree
E = N * DV
B = 512
T = 10
NCORES = 8
BL = B // NCORES  # 64
PB = 128
GB_ = M // PB     # 32 blocks per slot plane
CB = 4            # blocks per compute / scatter / gather chunk
NCK = GB_ // CB   # 8 chunks
CM = M // NCK     # 512 tokens per chunk

F32 = mybir.dt.float32
I32 = mybir.dt.int32
I16 = mybir.dt.int16
ALU = mybir.AluOpType
ACTF = mybir.ActivationFunctionType


def _derive_graph(edge_v: np.ndarray, edge_c: np.ndarray):
    """Host-side index derivation (layered 6-regular/3-regular graph)."""
    edge_v = np.asarray(edge_v, dtype=np.int64)
    edge_c = np.asarray(edge_c, dtype=np.int64)
    assert edge_v.shape == (E,) and edge_c.shape == (E,)

    order = np.argsort(edge_c, kind="stable")
    assert (edge_c[order] == np.repeat(np.arange(M), DC)).all(), (
        "graph is not 6-regular on checks"
    )
    slot_edge = order.reshape(M, DC).T.copy()  # [DC, M] edge id at (slot j, check c)

    j_of_e = np.empty(E, dtype=np.int64)
    c_of_e = np.empty(E, dtype=np.int64)
    for j in range(DC):
        j_of_e[slot_edge[j]] = j
        c_of_e[slot_edge[j]] = np.arange(M)

    # each variable must have exactly one edge in slots {0,1}, {2,3}, {4,5}
    layer_of_e = j_of_e // 2
    ve = np.full((N, 3), -1, dtype=np.int64)
    for lay in range(3):
        sel = np.where(layer_of_e == lay)[0]
        vs = edge_v[sel]
        assert len(np.unique(vs)) == N, f"layer {lay} is not a permutation"
        ve[vs, lay] = sel
    assert (ve >= 0).all()

    # storage: check c <-> (p = c % 128, g = c // 128)
    p_of_c = c_of_e % PB
    g_of_c = c_of_e // PB

    # u-row of a variable = its layer-0 edge position: (parity j0, p0, g0)
    e0 = ve[:, 0]
    pi_of_v = j_of_e[e0]
    p0_of_v = p_of_c[e0]
    g0_of_v = g_of_c[e0]
    fr_of_v = pi_of_v * M + p0_of_v * GB_ + g0_of_v       # u/llr DRAM row
    code_of_v = ((g0_of_v * 2 + pi_of_v) << 7) | p0_of_v  # SBUF scatter code

    # crossing-1 scatter lists: plane j (2..5), list pos = check c
    ixc1 = np.empty((4, M), dtype=np.int16)
    # crossing-2 / init gather lists: plane j, list pos = c: udram row of v(j,c)
    ixu = np.empty((4, M), dtype=np.int16)
    for j in range(2, DC):
        v = edge_v[slot_edge[j]]
        ixc1[j - 2] = code_of_v[v]
        ixu[j - 2] = fr_of_v[v]

    vid_of_fr = np.empty(N, dtype=np.int64)
    vid_of_fr[fr_of_v] = np.arange(N)
    return ixc1, ixu, vid_of_fr


def _wrap_idx(idx_m: np.ndarray) -> np.ndarray:
    """index layout: list position k at [k%16, k//16], replicated x8."""
    w = idx_m.reshape(-1, 16).T
    return np.tile(w, (PB // 16, 1)).copy()


def _build_program(alpha: np.ndarray, beta: np.ndarray) -> bacc.Bacc:
    nc = bacc.Bacc(num_swdge_queues=4)

    llr_t = nc.dram_tensor("llr_t", [N, BL], F32, kind="ExternalInput").ap()
    ixc1_d = nc.dram_tensor("ixc1", [4, PB, M // 16], I16,
                            kind="ExternalInput").ap()
    ixu_d = nc.dram_tensor("ixu", [4, PB, M // 16], I16, kind="ExternalInput").ap()
    post_d = nc.dram_tensor("post", [2, PB, GB_, BL], F32, kind="ExternalOutput").ap()
    bits_d = nc.dram_tensor("bits", [2, PB, GB_, BL], I32, kind="ExternalOutput").ap()
    udrs = [
        nc.dram_tensor("uda", [N, BL], F32).ap(),
        nc.dram_tensor("udb", [N, BL], F32).ap(),
    ]
    udrv = [u.rearrange("(pi p g) e -> p pi g e", pi=2, p=PB) for u in udrs]
    llrv = llr_t.rearrange("(pi p g) e -> p pi g e", pi=2, p=PB)
    bitv = bits_d.rearrange("pi p g e -> p pi g e")
    postv = post_d.rearrange("pi p g e -> p pi g e")

    QN = [0]

    def qn():
        q = QN[0] % 4
        QN[0] += 1
        return q

    dma_sems = [nc.alloc_semaphore(f"swdge_dma_q{q}") for q in range(4)]

    with tile.TileContext(nc) as tc:
        with (
            tc.tile_pool(name="persist", bufs=1) as pp,
            tc.tile_pool(name="tmp", bufs=1) as tp,
            tc.tile_pool(name="ut", bufs=1) as utp,
            tc.tile_pool(name="ps", bufs=1, space="PSUM") as psp,
        ):
            ixc1 = [pp.tile([PB, M // 16], I16, tag=f"ixc{j}", name=f"ixc{j}")
                    for j in range(4)]
            ixu = [pp.tile([PB, M // 16], I16, tag=f"ixu{i}", name=f"ixu{i}")
                   for i in range(4)]
            for j in range(4):
                nc.sync.dma_start(ixc1[j][:], ixc1_d[j])
                nc.sync.dma_start(ixu[j][:], ixu_d[j])

            # u (slots 0,1) / gathered u (slots 2..5); x = u - C' (pre-scaled)
            U = pp.tile([PB, DC, GB_, BL], F32, tag="u", name="u")
            # C' = alpha_t * c2v (alpha folded at c2v compute; 1.0 on last)
            C = pp.tile([PB, DC, GB_, BL], F32, tag="c", name="c")
            # scatter accumulator pairs: SA[0] <- planes 2,3 (init = llr),
            # SA[1] <- planes 4,5 (init = 0); u = SA[0] + SA[1] + C'01
            SA = [
                [pp.tile([PB, GB_, BL], F32, tag=f"sa{g}{pi}", name=f"sa{g}{pi}")
                 for pi in range(2)]
                for g in range(2)
            ]

            # init: U slots 0,1 = llr (u-row order); slots 2..5 gathered
            # inside iteration 0's check loop (interleaved, src=llr_t)
            nc.sync.dma_start(
                U[:, 0:2, :, :], llrv
            )
            for pi in range(2):
                nc.sync.dma_start(SA[0][pi][:], llrv[:, pi, :, :])
                nc.vector.memset(SA[1][pi][:], 0.0)

            def check_chunk(t, ck, sfold):
                """min-sum check update for compute chunk ck (CB blocks)."""
                b0 = ck * CB
                S1 = CB * BL
                blk = slice(b0, b0 + CB)
                cs = C[:, :, blk, :]
                us = U[:, :, blk, :]
                if t > 0:
                    xt = psp.tile([PB, DC, CB, BL], F32, tag="x", name="xt")
                    nc.vector.tensor_tensor(xt[:], us, cs, ALU.subtract)
                    xs = xt[:]
                else:
                    xs = us
                mg = tp.tile([PB, DC, CB, BL], F32, tag="m", name="mg")
                sg = tp.tile([PB, DC, CB, BL], F32, tag="s", name="sg")
                nc.scalar.activation(mg[:], xs, ACTF.Abs)
                nc.scalar.activation(sg[:], xs, ACTF.Sign)
                pp3 = tp.tile([PB, 3, CB, BL], F32, tag="p3", name="pp3")
                qq3 = tp.tile([PB, 3, CB, BL], F32, tag="q3", name="qq3")
                sp3 = tp.tile([PB, 3, CB, BL], F32, tag="sp3", name="sp3")
                bsp = tp.tile([PB, CB, BL], F32, tag="bsp", name="bsp")
                ex = psp.tile([PB, DC, CB, BL], F32, tag="e", name="ex")
                # pair mins / pair sign-products (even x odd slots, strided)
                nc.vector.tensor_tensor(pp3[:], mg[:, 0::2], mg[:, 1::2], ALU.min)
                nc.vector.tensor_tensor(sp3[:], sg[:, 0::2], sg[:, 1::2], ALU.mult)
                # leave-one-pair-out mins
                nc.vector.tensor_tensor(qq3[:, 0], pp3[:, 1], pp3[:, 2], ALU.min)
                nc.vector.tensor_tensor(qq3[:, 1], pp3[:, 0], pp3[:, 2], ALU.min)
                nc.vector.tensor_tensor(qq3[:, 2], pp3[:, 0], pp3[:, 1], ALU.min)
                # leave-one-out min: E[j] = min(M[partner(j)], Q[j//2])
                mv = mg[:]
                msw = bass.AP(
                    mv.tensor, mv.offset + S1,
                    [mv.ap[0], [2 * S1, 3], [-S1, 2], [1, S1]],
                )
                qb = (qq3[:].rearrange("p a b e -> p a (b e)")[:, :, None, :]
                      .to_broadcast([PB, 3, 2, S1]))
                nc.vector.tensor_tensor(
                    ex[:].rearrange("p (a b) c e -> p a b (c e)", a=3), msw, qb,
                    ALU.min,
                )
                # total sign product * (alpha_t * beta_t) [alpha pre-folded]
                nc.vector.scalar_tensor_tensor(
                    bsp[:], sp3[:, 0], float(sfold), sp3[:, 1], ALU.mult, ALU.mult
                )
                nc.vector.tensor_tensor(bsp[:], bsp[:], sp3[:, 2], ALU.mult)
                # C' = alpha*c2v = (sign * fold*sprod) * exclmin
                bb = bsp[:, None, :, :].to_broadcast([PB, DC, CB, BL])
                nc.vector.tensor_tensor(sg[:], sg[:], bb, ALU.mult)
                nc.vector.tensor_tensor(cs, sg[:], ex[:], ALU.mult)

            LEAD = 2

            for t in range(T):
                last = t == T - 1
                # C' = (alpha_t*beta_t)*... ; on the last iteration posterior
                # uses raw c2v sums, so only beta is folded.
                sfold = float(beta[t]) * (1.0 if last else float(alpha[t]))
                udt, udvt = udrs[t % 2], udrv[t % 2]
                # source for this iteration's input gathers (u of t-1)
                gsrc = llr_t if t == 0 else udrs[(t - 1) % 2]

                def gather_wave(ck, gsrc=gsrc):
                    hs = slice(ck * CB, (ck + 1) * CB)
                    ls = slice(ck * (CM // 16), (ck + 1) * (CM // 16))
                    for j in range(4):
                        nc.gpsimd.dma_gather(
                            U[:, 2 + j, hs, :], gsrc, ixu[j][:, ls],
                            CM, CM, BL,
                            single_packet=False, queue_num=qn(),
                        )

                # --- check phase; input gather waves emitted LEAD chunks
                # ahead of their consumer, scatter waves after each chunk so
                # the GpSimd engine alternates between the two.  Scatter
                # waves are graduated (2-block at the end) so the final
                # chain links are short and u-compute starts sooner. ---
                for w in range(LEAD):
                    gather_wave(w)
                for ck in range(NCK):
                    check_chunk(t, ck, sfold)
                    if ck + LEAD < NCK:
                        gather_wave(ck + LEAD)
                    if ck < NCK - 2:
                        waves = [(ck * CB, CB)]
                    else:
                        waves = [(ck * CB, CB // 2), (ck * CB + CB // 2, CB // 2)]
                    for b0, nb in waves:
                        nidx = nb * PB
                        ils = slice(b0 * PB // 16, (b0 * PB + nidx) // 16)
                        for j in range(4):
                            nc.gpsimd.dma_scatter_add(
                                SA[j // 2][0][:],
                                C[:, 2 + j, b0 : b0 + nb, :],
                                ixc1[j][:, ils],
                                nidx, nidx, BL,
                                single_packet=True,
                                queue_num=j // 2,
                                sbuf_tokens_per_rank=PB,
                                parity_reg=0,
                                out_ap_other=SA[j // 2][1][:],
                            )

                # --- u-compute per (parity, 16-block) chunk:
                # u = SA0 (llr + l1 sums) + SA1 (l2 sums) + C'01 ---
                for pi in range(2):
                    for h in range(2):
                        hs = slice(h * 16, (h + 1) * 16)
                        ua = utp.tile([PB, 16, BL], F32, tag="ua", name="ua")
                        nc.vector.tensor_tensor(
                            ua[:], SA[0][pi][:, hs, :], SA[1][pi][:, hs, :], ALU.add
                        )
                        up = U[:, pi, hs, :]
                        nc.vector.tensor_tensor(
                            up, ua[:], C[:, pi, hs, :], ALU.add
                        )
                        if not last:
                            nc.sync.dma_start(udvt[:, pi, hs, :], up)
                        else:
                            # posterior = llr + s ; bits = posterior < 0
                            bt = ua[:].bitcast(I32)
                            nc.vector.tensor_scalar(
                                bt, up, 0.0, None, ALU.is_lt
                            )
                            nc.sync.dma_start(postv[:, pi, hs, :], up)
                            nc.sync.dma_start(bitv[:, pi, hs, :], bt)
                # re-init the accumulators for the next iteration: SA[0]
                # reloads llr (affine DMA), SA[1] zeroes on the Scalar engine
                if not last:
                    for pi in range(2):
                        nc.sync.dma_start(SA[0][pi][:], llrv[:, pi, :, :])
                        nc.scalar.activation(
                            SA[1][pi][:], SA[1][pi][:], ACTF.Copy, scale=0.0
                        )

    nc.compile()
    return nc


def _prepare(llr, edge_v, edge_c, beta, alpha):
    ixc1, ixu, vid_of_fr = _derive_graph(edge_v, edge_c)
    ixc1w = np.stack([_wrap_idx(ixc1[j]) for j in range(4)])
    ixuw = np.stack([_wrap_idx(ixu[i]) for i in range(4)])

    llr = np.asarray(llr, dtype=np.float32)
    in_maps = []
    for k in range(NCORES):
        llr_t = np.ascontiguousarray(llr[k * BL: (k + 1) * BL, vid_of_fr].T)
        in_maps.append({"llr_t": llr_t, "ixc1": ixc1w, "ixu": ixuw})
    return in_maps, vid_of_fr


def _assemble(results, vid_of_fr):
    posterior = np.empty((B, N), dtype=np.float32)
    bits = np.empty((B, N), dtype=np.int32)
    for k in range(NCORES):
        pd = results[k]["post"].reshape(N, BL)  # row = pi*4096 + p*32 + g
        bd = results[k]["bits"].reshape(N, BL)
        posterior[k * BL: (k + 1) * BL, vid_of_fr] = pd.T
        bits[k * BL: (k + 1) * BL, vid_of_fr] = bd.T
    return bits, posterior


def _run(llr, edge_v, edge_c, beta, alpha, trace=False, tmpdir=None):
    in_maps, vid_of_fr = _prepare(llr, edge_v, edge_c, beta, alpha)
    nc = _build_program(np.asarray(alpha, np.float32), np.asarray(beta, np.float32))
    res = run_bass_kernel_spmd(
        nc, in_maps, list(range(NCORES)), trace=trace, tmpdir=tmpdir
    )
    return _assemble(res.results, vid_of_fr), res


def kernel(llr, edge_v, edge_c, beta, alpha):
    (bits, posterior), _ = _run(llr, edge_v, edge_c, beta, alpha, trace=False)
    return bits, posterior


# revision 30
# speedup vs baseline: 1.4747x; 1.4747x over previous
"""Trainium2 Bass kernel for the neural 2D min-sum LDPC decoder problem.

Strategy (v5)
-------------
Data-parallel over the batch: B=512 codewords, 64 per NeuronCore (8 cores).
Per core, per-edge state lives in SBUF with the graph on the partition axis
(check c <-> partition c%128, block c//128) and the 64-batch on the free
axis (256B rows).  Variables are relabeled by their slot-{0,1} (layer-0)
position so u / llr storage row = (parity, check-row) of the layer-0 edge.

Both per-iteration crossings pipeline with compute at 4-block granularity:

  crossing 1 (c2v -> per-variable sums): SBUF->SBUF dma_scatter_add in
      parity-split CCE mode (sbuf_tokens_per_rank=128).  Slot plane 2+j
      scatter-adds into its own accumulator pair SA[j] on queue j (4
      independent WAW chains ride 4 SWDGE queues); a 512-descriptor wave
      fires after every check compute chunk, so the chains drain in
      lockstep with compute.  dest code = ((g*2+parity)<<7) | p.
  u-compute   u = llr + alpha*(SA0+SA1+SA2+SA3 + c2v_l0), llr streamed
      from DRAM; u written to udram (affine HWDGE).
  crossing 2 (u -> slot positions 2..5): destination-chunked HBM gathers
      from udram in 512-descriptor waves; wave k unblocks check chunk k of
      the next iteration while later waves drain underneath its compute.

The SWDGE descriptor drain (~3ns/desc pipelined, ~12ns/desc on a WAW
chain) is the capacity limit: 32768 descriptors x 256B per iteration.
alpha/beta are baked as immediates (compiled after inputs are known).
"""

import sys

for _p in ("/opt/trn_rl_repo",):
    if _p not in sys.path:
        sys.path.insert(0, _p)

import numpy as np

import concourse.bass as bass
import concourse.bacc as bacc
import concourse.mybir as mybir
import concourse.tile as tile
from concourse.bass_utils import run_bass_kernel_spmd

N = 8192          # variable nodes
M = 4096          # check nodes
DC = 6            # check degree (slots)
DV = 3            # variable degree
E = N * DV
B = 512
T = 10
NCORES = 8
BL = B // NCORES  # 64
PB = 128
GB_ = M // PB     # 32 blocks per slot plane
CB = 4            # blocks per compute / scatter / gather chunk
NCK = GB_ // CB   # 8 chunks
CM = M // NCK     # 512 tokens per chunk

F32 = mybir.dt.float32
I32 = mybir.dt.int32
I16 = mybir.dt.int16
ALU = mybir.AluOpType
ACTF = mybir.ActivationFunctionType


def _derive_graph(edge_v: np.ndarray, edge_c: np.ndarray):
    """Host-side index derivation (layered 6-regular/3-regular graph)."""
    edge_v = np.asarray(edge_v, dtype=np.int64)
    edge_c = np.asarray(edge_c, dtype=np.int64)
    assert edge_v.shape == (E,) and edge_c.shape == (E,)

    order = np.argsort(edge_c, kind="stable")
    assert (edge_c[order] == np.repeat(np.arange(M), DC)).all(), (
        "graph is not 6-regular on checks"
    )
    slot_edge = order.reshape(M, DC).T.copy()  # [DC, M] edge id at (slot j, check c)

    j_of_e = np.empty(E, dtype=np.int64)
    c_of_e = np.empty(E, dtype=np.int64)
    for j in range(DC):
        j_of_e[slot_edge[j]] = j
        c_of_e[slot_edge[j]] = np.arange(M)

    # each variable must have exactly one edge in slots {0,1}, {2,3}, {4,5}
    layer_of_e = j_of_e // 2
    ve = np.full((N, 3), -1, dtype=np.int64)
    for lay in range(3):
        sel = np.where(layer_of_e == lay)[0]
        vs = edge_v[sel]
        assert len(np.unique(vs)) == N, f"layer {lay} is not a permutation"
        ve[vs, lay] = sel
    assert (ve >= 0).all()

    # storage: check c <-> (p = c % 128, g = c // 128)
    p_of_c = c_of_e % PB
    g_of_c = c_of_e // PB

    # u-row of a variable = its layer-0 edge position: (parity j0, p0, g0)
    e0 = ve[:, 0]
    pi_of_v = j_of_e[e0]
    p0_of_v = p_of_c[e0]
    g0_of_v = g_of_c[e0]
    fr_of_v = pi_of_v * M + p0_of_v * GB_ + g0_of_v       # u/llr DRAM row
    code_of_v = ((g0_of_v * 2 + pi_of_v) << 7) | p0_of_v  # SBUF scatter code

    # crossing-1 scatter lists: plane j (2..5), list pos = check c
    ixc1 = np.empty((4, M), dtype=np.int16)
    # crossing-2 / init gather lists: plane j, list pos = c: udram row of v(j,c)
    ixu = np.empty((4, M), dtype=np.int16)
    for j in range(2, DC):
        v = edge_v[slot_edge[j]]
        ixc1[j - 2] = code_of_v[v]
        ixu[j - 2] = fr_of_v[v]

    vid_of_fr = np.empty(N, dtype=np.int64)
    vid_of_fr[fr_of_v] = np.arange(N)
    return ixc1, ixu, vid_of_fr


def _wrap_idx(idx_m: np.ndarray) -> np.ndarray:
    """index layout: list position k at [k%16, k//16], replicated x8."""
    w = idx_m.reshape(-1, 16).T
    return np.tile(w, (PB // 16, 1)).copy()


def _build_program(alpha: np.ndarray, beta: np.ndarray) -> bacc.Bacc:
    nc = bacc.Bacc(num_swdge_queues=4)

    llr_t = nc.dram_tensor("llr_t", [N, BL], F32, kind="ExternalInput").ap()
    ixc1_d = nc.dram_tensor("ixc1", [4, PB, M // 16], I16,
                            kind="ExternalInput").ap()
    ixu_d = nc.dram_tensor("ixu", [4, PB, M // 16], I16, kind="ExternalInput").ap()
    post_d = nc.dram_tensor("post", [2, PB, GB_, BL], F32, kind="ExternalOutput").ap()
    bits_d = nc.dram_tensor("bits", [2, PB, GB_, BL], I32, kind="ExternalOutput").ap()
    udrs = [
        nc.dram_tensor("uda", [N, BL], F32).ap(),
        nc.dram_tensor("udb", [N, BL], F32).ap(),
    ]
    udrv = [u.rearrange("(pi p g) e -> p pi g e", pi=2, p=PB) for u in udrs]
    llrv = llr_t.rearrange("(pi p g) e -> p pi g e", pi=2, p=PB)
    bitv = bits_d.rearrange("pi p g e -> p pi g e")
    postv = post_d.rearrange("pi p g e -> p pi g e")

    QN = [0]

    def qn():
        q = QN[0] % 4
        QN[0] += 1
        return q

    dma_sems = [nc.alloc_semaphore(f"swdge_dma_q{q}") for q in range(4)]

    with tile.TileContext(nc) as tc:
        with (
            tc.tile_pool(name="persist", bufs=1) as pp,
            tc.tile_pool(name="tmp", bufs=1) as tp,
            tc.tile_pool(name="ut", bufs=1) as utp,
            tc.tile_pool(name="ps", bufs=1, space="PSUM") as psp,
        ):
            ixc1 = [pp.tile([PB, M // 16], I16, tag=f"ixc{j}", name=f"ixc{j}")
                    for j in range(4)]
            ixu = [pp.tile([PB, M // 16], I16, tag=f"ixu{i}", name=f"ixu{i}")
                   for i in range(4)]
            for j in range(4):
                nc.sync.dma_start(ixc1[j][:], ixc1_d[j])
                nc.sync.dma_start(ixu[j][:], ixu_d[j])

            # u (slots 0,1) / gathered u (slots 2..5); x = u - C' (pre-scaled)
            U = pp.tile([PB, DC, GB_, BL], F32, tag="u", name="u")
            # C' = alpha_t * c2v (alpha folded at c2v compute; 1.0 on last)
            C = pp.tile([PB, DC, GB_, BL], F32, tag="c", name="c")
            # per-plane scatter accumulator pairs (4 independent WAW chains
            # on 4 queues); SA[0] is llr-initialized, SA[1..3] zeroed, so
            # u = SA0 + SA1 + SA2 + SA3 + C'01
            SA = [
                [pp.tile([PB, GB_, BL], F32, tag=f"sa{g}{pi}", name=f"sa{g}{pi}")
                 for pi in range(2)]
                for g in range(4)
            ]

            # init: U slots 0,1 = llr (u-row order); slots 2..5 gathered
            # inside iteration 0's check loop (interleaved, src=llr_t)
            nc.sync.dma_start(
                U[:, 0:2, :, :], llrv
            )
            for pi in range(2):
                nc.sync.dma_start(SA[0][pi][:], llrv[:, pi, :, :])
                for g in range(1, 4):
                    nc.vector.memset(SA[g][pi][:], 0.0)

            def check_chunk(t, ck, sfold):
                """min-sum check update for compute chunk ck (CB blocks)."""
                b0 = ck * CB
                S1 = CB * BL
                blk = slice(b0, b0 + CB)
                cs = C[:, :, blk, :]
                us = U[:, :, blk, :]
                if t > 0:
                    xt = psp.tile([PB, DC, CB, BL], F32, tag="x", name="xt")
                    nc.vector.tensor_tensor(xt[:], us, cs, ALU.subtract)
                    xs = xt[:]
                else:
                    xs = us
                mg = tp.tile([PB, DC, CB, BL], F32, tag="m", name="mg")
                sg = tp.tile([PB, DC, CB, BL], F32, tag="s", name="sg")
                nc.scalar.activation(mg[:], xs, ACTF.Abs)
                nc.scalar.activation(sg[:], xs, ACTF.Sign)
                pp3 = tp.tile([PB, 3, CB, BL], F32, tag="p3", name="pp3")
                qq3 = tp.tile([PB, 3, CB, BL], F32, tag="q3", name="qq3")
                sp3 = tp.tile([PB, 3, CB, BL], F32, tag="sp3", name="sp3")
                bsp = tp.tile([PB, CB, BL], F32, tag="bsp", name="bsp")
                ex = psp.tile([PB, DC, CB, BL], F32, tag="e", name="ex")
                # pair mins / pair sign-products (even x odd slots, strided)
                nc.vector.tensor_tensor(pp3[:], mg[:, 0::2], mg[:, 1::2], ALU.min)
                nc.vector.tensor_tensor(sp3[:], sg[:, 0::2], sg[:, 1::2], ALU.mult)
                # leave-one-pair-out mins
                nc.vector.tensor_tensor(qq3[:, 0], pp3[:, 1], pp3[:, 2], ALU.min)
                nc.vector.tensor_tensor(qq3[:, 1], pp3[:, 0], pp3[:, 2], ALU.min)
                nc.vector.tensor_tensor(qq3[:, 2], pp3[:, 0], pp3[:, 1], ALU.min)
                # leave-one-out min: E[j] = min(M[partner(j)], Q[j//2])
                mv = mg[:]
                msw = bass.AP(
                    mv.tensor, mv.offset + S1,
                    [mv.ap[0], [2 * S1, 3], [-S1, 2], [1, S1]],
                )
                qb = (qq3[:].rearrange("p a b e -> p a (b e)")[:, :, None, :]
                      .to_broadcast([PB, 3, 2, S1]))
                nc.vector.tensor_tensor(
                    ex[:].rearrange("p (a b) c e -> p a b (c e)", a=3), msw, qb,
                    ALU.min,
                )
                # total sign product * (alpha_t * beta_t) [alpha pre-folded]
                nc.vector.scalar_tensor_tensor(
                    bsp[:], sp3[:, 0], float(sfold), sp3[:, 1], ALU.mult, ALU.mult
                )
                nc.vector.tensor_tensor(bsp[:], bsp[:], sp3[:, 2], ALU.mult)
                # C' = alpha*c2v = (sign * fold*sprod) * exclmin
                bb = bsp[:, None, :, :].to_broadcast([PB, DC, CB, BL])
                nc.vector.tensor_tensor(sg[:], sg[:], bb, ALU.mult)
                nc.vector.tensor_tensor(cs, sg[:], ex[:], ALU.mult)

            LEAD = 2

            for t in range(T):
                last = t == T - 1
                # C' = (alpha_t*beta_t)*... ; on the last iteration posterior
                # uses raw c2v sums, so only beta is folded.
                sfold = float(beta[t]) * (1.0 if last else float(alpha[t]))
                udt, udvt = udrs[t % 2], udrv[t % 2]
                # source for this iteration's input gathers (u of t-1)
                gsrc = llr_t if t == 0 else udrs[(t - 1) % 2]

                def gather_wave(ck, gsrc=gsrc):
                    hs = slice(ck * CB, (ck + 1) * CB)
                    ls = slice(ck * (CM // 16), (ck + 1) * (CM // 16))
                    for j in range(4):
                        nc.gpsimd.dma_gather(
                            U[:, 2 + j, hs, :], gsrc, ixu[j][:, ls],
                            CM, CM, BL,
                            single_packet=False, queue_num=qn(),
                        )

                # --- check phase; input gather waves emitted LEAD chunks
                # ahead of their consumer, scatter waves after each chunk so
                # the GpSimd engine alternates between the two.  Scatter
                # waves are graduated (2-block at the end) so the final
                # chain links are short and u-compute starts sooner. ---
                for w in range(LEAD):
                    gather_wave(w)
                for ck in range(NCK):
                    check_chunk(t, ck, sfold)
                    if ck + LEAD < NCK:
                        gather_wave(ck + LEAD)
                    if ck < NCK - 2:
                        waves = [(ck * CB, CB)]
                    else:
                        waves = [(ck * CB, CB // 2), (ck * CB + CB // 2, CB // 2)]
                    for b0, nb in waves:
                        nidx = nb * PB
                        ils = slice(b0 * PB // 16, (b0 * PB + nidx) // 16)
                        for j in range(4):
                            nc.gpsimd.dma_scatter_add(
                                SA[j][0][:],
                                C[:, 2 + j, b0 : b0 + nb, :],
                                ixc1[j][:, ils],
                                nidx, nidx, BL,
                                single_packet=True,
                                queue_num=j,
                                sbuf_tokens_per_rank=PB,
                                parity_reg=0,
                                out_ap_other=SA[j][1][:],
                            )

                # --- u-compute per (parity, 16-block) chunk:
                # u = (SA0+SA1) + (SA2+SA3) + C'01 (llr baked into SA0) ---
                for pi in range(2):
                    for h in range(2):
                        hs = slice(h * 16, (h + 1) * 16)
                        ua = utp.tile([PB, 16, BL], F32, tag="ua", name="ua")
                        ub = utp.tile([PB, 16, BL], F32, tag="ub", name="ub")
                        nc.vector.tensor_tensor(
                            ua[:], SA[0][pi][:, hs, :], SA[1][pi][:, hs, :], ALU.add
                        )
                        nc.vector.tensor_tensor(
                            ub[:], SA[2][pi][:, hs, :], SA[3][pi][:, hs, :], ALU.add
                        )
                        nc.vector.tensor_tensor(ua[:], ua[:], ub[:], ALU.add)
                        up = U[:, pi, hs, :]
                        nc.vector.tensor_tensor(
                            up, ua[:], C[:, pi, hs, :], ALU.add
                        )
                        if not last:
                            nc.sync.dma_start(udvt[:, pi, hs, :], up)
                        else:
                            # posterior = llr + s ; bits = posterior < 0
                            bt = ua[:].bitcast(I32)
                            nc.vector.tensor_scalar(
                                bt, up, 0.0, None, ALU.is_lt
                            )
                            nc.sync.dma_start(postv[:, pi, hs, :], up)
                            nc.sync.dma_start(bitv[:, pi, hs, :], bt)
                # re-init the accumulators for the next iteration: SA[0]
                # reloads llr (affine DMA), SA[1..3] zero on the Scalar engine
                if not last:
                    for pi in range(2):
                        nc.sync.dma_start(SA[0][pi][:], llrv[:, pi, :, :])
                        for g in range(1, 4):
                            nc.scalar.activation(
                                SA[g][pi][:], SA[g][pi][:], ACTF.Copy, scale=0.0
                            )

    nc.compile()
    return nc


def _prepare(llr, edge_v, edge_c, beta, alpha):
    ixc1, ixu, vid_of_fr = _derive_graph(edge_v, edge_c)
    ixc1w = np.stack([_wrap_idx(ixc1[j]) for j in range(4)])
    ixuw = np.stack([_wrap_idx(ixu[i]) for i in range(4)])

    llr = np.asarray(llr, dtype=np.float32)
    in_maps = []
    for k in range(NCORES):
        llr_t = np.ascontiguousarray(llr[k * BL: (k + 1) * BL, vid_of_fr].T)
        in_maps.append({"llr_t": llr_t, "ixc1": ixc1w, "ixu": ixuw})
    return in_maps, vid_of_fr


def _assemble(results, vid_of_fr):
    posterior = np.empty((B, N), dtype=np.float32)
    bits = np.empty((B, N), dtype=np.int32)
    for k in range(NCORES):
        pd = results[k]["post"].reshape(N, BL)  # row = pi*4096 + p*32 + g
        bd = results[k]["bits"].reshape(N, BL)
        posterior[k * BL: (k + 1) * BL, vid_of_fr] = pd.T
        bits[k * BL: (k + 1) * BL, vid_of_fr] = bd.T
    return bits, posterior


def _run(llr, edge_v, edge_c, beta, alpha, trace=False, tmpdir=None):
    in_maps, vid_of_fr = _prepare(llr, edge_v, edge_c, beta, alpha)
    nc = _build_program(np.asarray(alpha, np.float32), np.asarray(beta, np.float32))
    res = run_bass_kernel_spmd(
        nc, in_maps, list(range(NCORES)), trace=trace, tmpdir=tmpdir
    )
    return _assemble(res.results, vid_of_fr), res


def kernel(llr, edge_v, edge_c, beta, alpha):
    (bits, posterior), _ = _run(llr, edge_v, edge_c, beta, alpha, trace=False)
    return bits, posterior


# revision 31
# speedup vs baseline: 1.5054x; 1.0208x over previous
"""Trainium2 Bass kernel for the neural 2D min-sum LDPC decoder problem.

Strategy (v5)
-------------
Data-parallel over the batch: B=512 codewords, 64 per NeuronCore (8 cores).
Per core, per-edge state lives in SBUF with the graph on the partition axis
(check c <-> partition c%128, block c//128) and the 64-batch on the free
axis (256B rows).  Variables are relabeled by their slot-{0,1} (layer-0)
position so u / llr storage row = (parity, check-row) of the layer-0 edge.

Both per-iteration crossings pipeline with compute at 4-block granularity:

  crossing 1 (c2v -> per-variable sums): SBUF->SBUF dma_scatter_add in
      parity-split CCE mode (sbuf_tokens_per_rank=128).  Slot plane 2+j
      scatter-adds into its own accumulator pair SA[j] on queue j (4
      independent WAW chains ride 4 SWDGE queues); a 512-descriptor wave
      fires after every check compute chunk, so the chains drain in
      lockstep with compute.  dest code = ((g*2+parity)<<7) | p.
  u-compute   u = llr + alpha*(SA0+SA1+SA2+SA3 + c2v_l0), llr streamed
      from DRAM; u written to udram (affine HWDGE).
  crossing 2 (u -> slot positions 2..5): destination-chunked HBM gathers
      from udram in 512-descriptor waves; wave k unblocks check chunk k of
      the next iteration while later waves drain underneath its compute.

The SWDGE descriptor drain (~3ns/desc pipelined, ~12ns/desc on a WAW
chain) is the capacity limit: 32768 descriptors x 256B per iteration.
alpha/beta are baked as immediates (compiled after inputs are known).
"""

import sys

for _p in ("/opt/trn_rl_repo",):
    if _p not in sys.path:
        sys.path.insert(0, _p)

import numpy as np

import concourse.bass as bass
import concourse.bacc as bacc
import concourse.mybir as mybir
import concourse.tile as tile
from concourse.bass_utils import run_bass_kernel_spmd

N = 8192          # variable nodes
M = 4096          # check nodes
DC = 6            # check degree (slots)
DV = 3            # variable degree
E = N * DV
B = 512
T = 10
NCORES = 8
BL = B // NCORES  # 64
PB = 128
GB_ = M // PB     # 32 blocks per slot plane
CB = 4            # blocks per compute / scatter / gather chunk
NCK = GB_ // CB   # 8 chunks
CM = M // NCK     # 512 tokens per chunk

F32 = mybir.dt.float32
I32 = mybir.dt.int32
I16 = mybir.dt.int16
ALU = mybir.AluOpType
ACTF = mybir.ActivationFunctionType


def _derive_graph(edge_v: np.ndarray, edge_c: np.ndarray):
    """Host-side index derivation (layered 6-regular/3-regular graph)."""
    edge_v = np.asarray(edge_v, dtype=np.int64)
    edge_c = np.asarray(edge_c, dtype=np.int64)
    assert edge_v.shape == (E,) and edge_c.shape == (E,)

    order = np.argsort(edge_c, kind="stable")
    assert (edge_c[order] == np.repeat(np.arange(M), DC)).all(), (
        "graph is not 6-regular on checks"
    )
    slot_edge = order.reshape(M, DC).T.copy()  # [DC, M] edge id at (slot j, check c)

    j_of_e = np.empty(E, dtype=np.int64)
    c_of_e = np.empty(E, dtype=np.int64)
    for j in range(DC):
        j_of_e[slot_edge[j]] = j
        c_of_e[slot_edge[j]] = np.arange(M)

    # each variable must have exactly one edge in slots {0,1}, {2,3}, {4,5}
    layer_of_e = j_of_e // 2
    ve = np.full((N, 3), -1, dtype=np.int64)
    for lay in range(3):
        sel = np.where(layer_of_e == lay)[0]
        vs = edge_v[sel]
        assert len(np.unique(vs)) == N, f"layer {lay} is not a permutation"
        ve[vs, lay] = sel
    assert (ve >= 0).all()

    # storage: check c <-> (p = c % 128, g = c // 128)
    p_of_c = c_of_e % PB
    g_of_c = c_of_e // PB

    # u-row of a variable = its layer-0 edge position: (parity j0, p0, g0)
    e0 = ve[:, 0]
    pi_of_v = j_of_e[e0]
    p0_of_v = p_of_c[e0]
    g0_of_v = g_of_c[e0]
    fr_of_v = pi_of_v * M + p0_of_v * GB_ + g0_of_v       # u/llr DRAM row
    code_of_v = ((g0_of_v * 2 + pi_of_v) << 7) | p0_of_v  # SBUF scatter code

    # crossing-1 scatter lists: plane j (2..5), list pos = check c
    ixc1 = np.empty((4, M), dtype=np.int16)
    # crossing-2 / init gather lists: plane j, list pos = c: udram row of v(j,c)
    ixu = np.empty((4, M), dtype=np.int16)
    for j in range(2, DC):
        v = edge_v[slot_edge[j]]
        ixc1[j - 2] = code_of_v[v]
        ixu[j - 2] = fr_of_v[v]

    vid_of_fr = np.empty(N, dtype=np.int64)
    vid_of_fr[fr_of_v] = np.arange(N)
    return ixc1, ixu, vid_of_fr


def _wrap_idx(idx_m: np.ndarray) -> np.ndarray:
    """index layout: list position k at [k%16, k//16], replicated x8."""
    w = idx_m.reshape(-1, 16).T
    return np.tile(w, (PB // 16, 1)).copy()


def _build_program(alpha: np.ndarray, beta: np.ndarray) -> bacc.Bacc:
    nc = bacc.Bacc(num_swdge_queues=4)

    llr_t = nc.dram_tensor("llr_t", [N, BL], F32, kind="ExternalInput").ap()
    ixc1_d = nc.dram_tensor("ixc1", [4, PB, M // 16], I16,
                            kind="ExternalInput").ap()
    ixu_d = nc.dram_tensor("ixu", [4, PB, M // 16], I16, kind="ExternalInput").ap()
    post_d = nc.dram_tensor("post", [2, PB, GB_, BL], F32, kind="ExternalOutput").ap()
    bits_d = nc.dram_tensor("bits", [2, PB, GB_, BL], I32, kind="ExternalOutput").ap()
    udrs = [
        nc.dram_tensor("uda", [N, BL], F32).ap(),
        nc.dram_tensor("udb", [N, BL], F32).ap(),
    ]
    udrv = [u.rearrange("(pi p g) e -> p pi g e", pi=2, p=PB) for u in udrs]
    llrv = llr_t.rearrange("(pi p g) e -> p pi g e", pi=2, p=PB)
    bitv = bits_d.rearrange("pi p g e -> p pi g e")
    postv = post_d.rearrange("pi p g e -> p pi g e")

    QN = [0]

    def qn():
        q = QN[0] % 4
        QN[0] += 1
        return q

    dma_sems = [nc.alloc_semaphore(f"swdge_dma_q{q}") for q in range(4)]

    with tile.TileContext(nc) as tc:
        with (
            tc.tile_pool(name="persist", bufs=1) as pp,
            tc.tile_pool(name="tmp", bufs=1) as tp,
            tc.tile_pool(name="ut", bufs=1) as utp,
            tc.tile_pool(name="ps", bufs=1, space="PSUM") as psp,
        ):
            ixc1 = [pp.tile([PB, M // 16], I16, tag=f"ixc{j}", name=f"ixc{j}")
                    for j in range(4)]
            ixu = [pp.tile([PB, M // 16], I16, tag=f"ixu{i}", name=f"ixu{i}")
                   for i in range(4)]
            for j in range(4):
                nc.sync.dma_start(ixc1[j][:], ixc1_d[j])
                nc.sync.dma_start(ixu[j][:], ixu_d[j])

            # u (slots 0,1) / gathered u (slots 2..5); x = u - C' (pre-scaled)
            U = pp.tile([PB, DC, GB_, BL], F32, tag="u", name="u")
            # C' = alpha_t * c2v (alpha folded at c2v compute; 1.0 on last)
            C = pp.tile([PB, DC, GB_, BL], F32, tag="c", name="c")
            # per-plane scatter accumulator pairs (4 independent WAW chains
            # on 4 queues); SA[0] is llr-initialized, SA[1..3] zeroed, so
            # u = SA0 + SA1 + SA2 + SA3 + C'01
            SA = [
                [pp.tile([PB, GB_, BL], F32, tag=f"sa{g}{pi}", name=f"sa{g}{pi}")
                 for pi in range(2)]
                for g in range(4)
            ]

            # init: U slots 0,1 = llr (u-row order); slots 2..5 gathered
            # inside iteration 0's check loop (interleaved, src=llr_t)
            nc.sync.dma_start(
                U[:, 0:2, :, :], llrv
            )
            for pi in range(2):
                nc.sync.dma_start(SA[0][pi][:], llrv[:, pi, :, :])
                for g in range(1, 4):
                    nc.vector.memset(SA[g][pi][:], 0.0)

            def check_chunk(t, ck, sfold):
                """min-sum check update for compute chunk ck (CB blocks)."""
                b0 = ck * CB
                S1 = CB * BL
                blk = slice(b0, b0 + CB)
                cs = C[:, :, blk, :]
                us = U[:, :, blk, :]
                if t > 0:
                    xt = psp.tile([PB, DC, CB, BL], F32, tag="x", name="xt")
                    nc.vector.tensor_tensor(xt[:], us, cs, ALU.subtract)
                    xs = xt[:]
                else:
                    xs = us
                mg = tp.tile([PB, DC, CB, BL], F32, tag="m", name="mg")
                sg = tp.tile([PB, DC, CB, BL], F32, tag="s", name="sg")
                nc.scalar.activation(mg[:], xs, ACTF.Abs)
                nc.scalar.activation(sg[:], xs, ACTF.Sign)
                pp3 = tp.tile([PB, 3, CB, BL], F32, tag="p3", name="pp3")
                qq3 = tp.tile([PB, 3, CB, BL], F32, tag="q3", name="qq3")
                sp3 = tp.tile([PB, 3, CB, BL], F32, tag="sp3", name="sp3")
                bsp = tp.tile([PB, CB, BL], F32, tag="bsp", name="bsp")
                ex = psp.tile([PB, DC, CB, BL], F32, tag="e", name="ex")
                # pair mins / pair sign-products (even x odd slots, strided)
                nc.vector.tensor_tensor(pp3[:], mg[:, 0::2], mg[:, 1::2], ALU.min)
                nc.vector.tensor_tensor(sp3[:], sg[:, 0::2], sg[:, 1::2], ALU.mult)
                # leave-one-pair-out mins
                nc.vector.tensor_tensor(qq3[:, 0], pp3[:, 1], pp3[:, 2], ALU.min)
                nc.vector.tensor_tensor(qq3[:, 1], pp3[:, 0], pp3[:, 2], ALU.min)
                nc.vector.tensor_tensor(qq3[:, 2], pp3[:, 0], pp3[:, 1], ALU.min)
                # leave-one-out min: E[j] = min(M[partner(j)], Q[j//2])
                mv = mg[:]
                msw = bass.AP(
                    mv.tensor, mv.offset + S1,
                    [mv.ap[0], [2 * S1, 3], [-S1, 2], [1, S1]],
                )
                qb = (qq3[:].rearrange("p a b e -> p a (b e)")[:, :, None, :]
                      .to_broadcast([PB, 3, 2, S1]))
                nc.vector.tensor_tensor(
                    ex[:].rearrange("p (a b) c e -> p a b (c e)", a=3), msw, qb,
                    ALU.min,
                )
                # total sign product * (alpha_t * beta_t) [alpha pre-folded]
                nc.vector.scalar_tensor_tensor(
                    bsp[:], sp3[:, 0], float(sfold), sp3[:, 1], ALU.mult, ALU.mult
                )
                nc.vector.tensor_tensor(bsp[:], bsp[:], sp3[:, 2], ALU.mult)
                # C' = alpha*c2v = (sign * fold*sprod) * exclmin
                bb = bsp[:, None, :, :].to_broadcast([PB, DC, CB, BL])
                nc.vector.tensor_tensor(sg[:], sg[:], bb, ALU.mult)
                nc.vector.tensor_tensor(cs, sg[:], ex[:], ALU.mult)

            LEAD = 2

            for t in range(T):
                last = t == T - 1
                # C' = (alpha_t*beta_t)*... ; on the last iteration posterior
                # uses raw c2v sums, so only beta is folded.
                sfold = float(beta[t]) * (1.0 if last else float(alpha[t]))
                udt, udvt = udrs[t % 2], udrv[t % 2]
                # source for this iteration's input gathers (u of t-1)
                gsrc = llr_t if t == 0 else udrs[(t - 1) % 2]

                def gather_wave(ck, gsrc=gsrc):
                    hs = slice(ck * CB, (ck + 1) * CB)
                    ls = slice(ck * (CM // 16), (ck + 1) * (CM // 16))
                    for j in range(4):
                        nc.gpsimd.dma_gather(
                            U[:, 2 + j, hs, :], gsrc, ixu[j][:, ls],
                            CM, CM, BL,
                            single_packet=False, queue_num=qn(),
                        )

                # --- check phase; input gather waves emitted LEAD chunks
                # ahead of their consumer, scatter waves after each chunk so
                # the GpSimd engine alternates between the two.  Scatter
                # waves are graduated (2-block at the end) so the final
                # chain links are short and u-compute starts sooner. ---
                for w in range(LEAD):
                    gather_wave(w)
                for ck in range(NCK):
                    check_chunk(t, ck, sfold)
                    if ck + LEAD < NCK:
                        gather_wave(ck + LEAD)
                    if ck < NCK - 2:
                        waves = [(ck * CB, CB)]
                    else:
                        waves = [(ck * CB, CB // 2), (ck * CB + CB // 2, CB // 2)]
                    for b0, nb in waves:
                        nidx = nb * PB
                        ils = slice(b0 * PB // 16, (b0 * PB + nidx) // 16)
                        for j in range(4):
                            nc.gpsimd.dma_scatter_add(
                                SA[j][0][:],
                                C[:, 2 + j, b0 : b0 + nb, :],
                                ixc1[j][:, ils],
                                nidx, nidx, BL,
                                single_packet=False,
                                queue_num=j,
                                sbuf_tokens_per_rank=PB,
                                parity_reg=0,
                                out_ap_other=SA[j][1][:],
                            )

                # --- u-compute per (parity, 16-block) chunk:
                # u = (SA0+SA1) + (SA2+SA3) + C'01 (llr baked into SA0) ---
                for pi in range(2):
                    for h in range(2):
                        hs = slice(h * 16, (h + 1) * 16)
                        ua = utp.tile([PB, 16, BL], F32, tag="ua", name="ua")
                        ub = utp.tile([PB, 16, BL], F32, tag="ub", name="ub")
                        nc.vector.tensor_tensor(
                            ua[:], SA[0][pi][:, hs, :], SA[1][pi][:, hs, :], ALU.add
                        )
                        nc.vector.tensor_tensor(
                            ub[:], SA[2][pi][:, hs, :], SA[3][pi][:, hs, :], ALU.add
                        )
                        nc.vector.tensor_tensor(ua[:], ua[:], ub[:], ALU.add)
                        up = U[:, pi, hs, :]
                        nc.vector.tensor_tensor(
                            up, ua[:], C[:, pi, hs, :], ALU.add
                        )
                        if not last:
                            nc.sync.dma_start(udvt[:, pi, hs, :], up)
                        else:
                            # posterior = llr + s ; bits = posterior < 0
                            bt = ua[:].bitcast(I32)
                            nc.vector.tensor_scalar(
                                bt, up, 0.0, None, ALU.is_lt
                            )
                            nc.sync.dma_start(postv[:, pi, hs, :], up)
                            nc.sync.dma_start(bitv[:, pi, hs, :], bt)
                # re-init the accumulators for the next iteration: SA[0]
                # reloads llr (affine DMA), SA[1..3] zero on the Scalar engine
                if not last:
                    for pi in range(2):
                        nc.sync.dma_start(SA[0][pi][:], llrv[:, pi, :, :])
                        for g in range(1, 4):
                            nc.scalar.activation(
                                SA[g][pi][:], SA[g][pi][:], ACTF.Copy, scale=0.0
                            )

    nc.compile()
    return nc


def _prepare(llr, edge_v, edge_c, beta, alpha):
    ixc1, ixu, vid_of_fr = _derive_graph(edge_v, edge_c)
    ixc1w = np.stack([_wrap_idx(ixc1[j]) for j in range(4)])
    ixuw = np.stack([_wrap_idx(ixu[i]) for i in range(4)])

    llr = np.asarray(llr, dtype=np.float32)
    in_maps = []
    for k in range(NCORES):
        llr_t = np.ascontiguousarray(llr[k * BL: (k + 1) * BL, vid_of_fr].T)
        in_maps.append({"llr_t": llr_t, "ixc1": ixc1w, "ixu": ixuw})
    return in_maps, vid_of_fr


def _assemble(results, vid_of_fr):
    posterior = np.empty((B, N), dtype=np.float32)
    bits = np.empty((B, N), dtype=np.int32)
    for k in range(NCORES):
        pd = results[k]["post"].reshape(N, BL)  # row = pi*4096 + p*32 + g
        bd = results[k]["bits"].reshape(N, BL)
        posterior[k * BL: (k + 1) * BL, vid_of_fr] = pd.T
        bits[k * BL: (k + 1) * BL, vid_of_fr] = bd.T
    return bits, posterior


def _run(llr, edge_v, edge_c, beta, alpha, trace=False, tmpdir=None):
    in_maps, vid_of_fr = _prepare(llr, edge_v, edge_c, beta, alpha)
    nc = _build_program(np.asarray(alpha, np.float32), np.asarray(beta, np.float32))
    res = run_bass_kernel_spmd(
        nc, in_maps, list(range(NCORES)), trace=trace, tmpdir=tmpdir
    )
    return _assemble(res.results, vid_of_fr), res


def kernel(llr, edge_v, edge_c, beta, alpha):
    (bits, posterior), _ = _run(llr, edge_v, edge_c, beta, alpha, trace=False)
    return bits, posterior
